# revision 15
# baseline (speedup 1.0000x reference)
"""Trainium2 Bass kernel for the BD3LM dense transformer.

Sharding: 8 cores = DP-2 over batch x 4-way sequence shard (256 tokens/core).
Activations feature-major [D(partitions), T(free)]. All GEMMs bf16 inputs with
f32 PSUM accumulation. Per-layer K/V AllGather (bf16) within each batch group.
Vocab head computed fully per core for its tokens (no cross-core softmax).
Loss assembled on host from logits + on-device sumexp.
"""

import os
import numpy as np
import ml_dtypes

import concourse.bass as bass
import concourse.tile as tile
from concourse import bacc, mybir
from concourse.bass_utils import run_bass_kernel_spmd

F32 = mybir.dt.float32
BF16 = mybir.dt.bfloat16
AF = mybir.ActivationFunctionType
AX = mybir.AxisListType

V, D, H, L, DF = 32000, 768, 12, 12, 3072
B, S, BS = 2, 1024, 16
HD = D // H            # 64
MASK_ID = V - 1
T = 256                # tokens per core
NT = T // 128          # 2 token tiles
ND = D // 128          # 6 feature tiles
NF = DF // 128         # 24
NH = H                 # 12 heads
NK = S // 128          # 8 key tiles
VA = H * (HD + 1)      # 780: V augmented with a ones column per head
N_CORES = 8
NVC = 63               # vocab chunks: 62x512 + 1x256
EPS = 1e-5

N_LAYERS = int(os.environ.get("BDK_LAYERS", str(L)))


def _vc_width(c):
    return 512 if c < 62 else 256


# ----------------------------------------------------------------------------
# program build
# ----------------------------------------------------------------------------

_NC_CACHE = {}


def build_nc(n_layers=N_LAYERS, include_pb=False):
    key = (n_layers, include_pb)
    if key in _NC_CACHE:
        return _NC_CACHE[key]
    nc = bacc.Bacc("TRN2", target_bir_lowering=False, debug=False,
                   num_devices=N_CORES)

    # ---- DRAM I/O ----
    x0 = nc.dram_tensor("x0", [128, ND, T], F32, kind="ExternalInput")
    wqk = nc.dram_tensor("wqk", [L, D, 2 * D], BF16, kind="ExternalInput")
    wv = nc.dram_tensor("wv", [L, D, VA], BF16, kind="ExternalInput")
    vbr = nc.dram_tensor("vbr", [L, 128, VA], F32, kind="ExternalInput")
    wo = nc.dram_tensor("wo", [L, D, D], BF16, kind="ExternalInput")
    w1 = nc.dram_tensor("w1", [L, D, DF], BF16, kind="ExternalInput")
    w2 = nc.dram_tensor("w2", [L, DF, D], BF16, kind="ExternalInput")
    pw = nc.dram_tensor("pw", [D, V], BF16, kind="ExternalInput")
    lnw = nc.dram_tensor("lnw", [128, L, 2, ND], F32, kind="ExternalInput")
    lnb = nc.dram_tensor("lnb", [128, L, 2, ND], F32, kind="ExternalInput")
    qkb = nc.dram_tensor("qkb", [128, L, 12], F32, kind="ExternalInput")
    aob = nc.dram_tensor("aob", [128, L, ND], F32, kind="ExternalInput")
    b1p = nc.dram_tensor("b1p", [128, L, NF], F32, kind="ExternalInput")
    b2p = nc.dram_tensor("b2p", [128, L, ND], F32, kind="ExternalInput")
    msk = nc.dram_tensor("msk", [128, NK, T], BF16, kind="ExternalInput")
    sel = nc.dram_tensor("sel", [H, D], BF16, kind="ExternalInput")
    if include_pb:
        pbr = nc.dram_tensor("pbr", [128, V], F32, kind="ExternalInput")

    logits = nc.dram_tensor("logits", [T, V], F32, kind="ExternalOutput")
    sumexp = nc.dram_tensor("sumexp", [NT, 128], F32, kind="ExternalOutput")
    xdbg = nc.dram_tensor("xdbg", [128, ND, T], F32, kind="ExternalOutput")

    KBYTES = 128 * ND * T          # elements in K^T slab (per-core local keys)
    VBYTES = NT * 128 * VA         # elements in V slab
    rg = [[0, 1, 2, 3], [4, 5, 6, 7]]

    with tile.TileContext(nc) as tc:
        with (
            tc.tile_pool(name="const", bufs=1) as cpool,
            tc.tile_pool(name="xp", bufs=1) as xpool,
        ):
            # persistent constants
            lnw_sb = cpool.tile([128, L, 2, ND], F32, tag="lnw")
            lnb_sb = cpool.tile([128, L, 2, ND], F32, tag="lnb")
            qkb_sb = cpool.tile([128, L, 12], F32, tag="qkb")
            aob_sb = cpool.tile([128, L, ND], F32, tag="aob")
            b1p_sb = cpool.tile([128, L, NF], F32, tag="b1p")
            b2p_sb = cpool.tile([128, L, ND], F32, tag="b2p")
            msk_sb = cpool.tile([128, NK, T], BF16, tag="msk")
            sel_sb = cpool.tile([H, D], BF16, tag="sel")
            ones_c = cpool.tile([128, 1], BF16, tag="ones_c")   # column of ones
            ones_r = cpool.tile([1, 128], BF16, tag="ones_r")   # row of ones
            nc.sync.dma_start(lnw_sb[:], lnw[:])
            nc.sync.dma_start(lnb_sb[:], lnb[:])
            nc.sync.dma_start(qkb_sb[:], qkb[:])
            nc.sync.dma_start(aob_sb[:], aob[:])
            nc.sync.dma_start(b1p_sb[:], b1p[:])
            nc.sync.dma_start(b2p_sb[:], b2p[:])
            nc.sync.dma_start(msk_sb[:], msk[:])
            nc.sync.dma_start(sel_sb[:], sel[:])
            nc.vector.memset(ones_c[:], 1.0)
            nc.vector.memset(ones_r[:], 1.0)

            x = xpool.tile([128, ND, T], F32, tag="x")
            nc.sync.dma_start(x[:], x0[:])

            with (
                tc.tile_pool(name="wts", bufs=1) as wpool,
                tc.tile_pool(name="wts2", bufs=3) as w2pool,
                tc.tile_pool(name="acts", bufs=2) as apool,
                tc.tile_pool(name="acts1", bufs=1) as a1pool,
                tc.tile_pool(name="ps", bufs=6, space="PSUM") as ps,
                tc.tile_pool(name="ps2", bufs=2, space="PSUM") as ps2,
                tc.tile_pool(name="dram", bufs=2, space="DRAM") as dpool,
            ):
                def layer_norm(l, which):
                    """x (f32 slab) -> normalized bf16 slab."""
                    ps_sum = ps2.tile([128, T], F32, tag="stat")
                    ps_sq = ps2.tile([128, T], F32, tag="stat")
                    for d in range(ND):
                        xb = apool.tile([128, T], BF16, tag="xb")
                        sq = apool.tile([128, T], BF16, tag="sq")
                        nc.scalar.activation(xb[:], x[:, d, :], AF.Copy)
                        nc.scalar.activation(sq[:], x[:, d, :], AF.Square)
                        nc.tensor.matmul(ps_sum[0:1, :], ones_c[:], xb[:],
                                         start=(d == 0), stop=(d == ND - 1))
                        nc.tensor.matmul(ps_sq[0:1, :], ones_c[:], sq[:],
                                         start=(d == 0), stop=(d == ND - 1))
                    st = apool.tile([1, 6, T], F32, tag="st")
                    m_ap = st[:, 0, :]
                    e2_ap = st[:, 1, :]
                    msq = st[:, 2, :]
                    vv = st[:, 3, :]
                    rr = st[:, 4, :]
                    rstd_f = st[:, 5, :]
                    nc.vector.tensor_scalar_mul(m_ap, ps_sum[0:1, :], 1.0 / D)
                    nc.vector.tensor_scalar_mul(e2_ap, ps_sq[0:1, :], 1.0 / D)
                    nc.vector.tensor_mul(msq, m_ap, m_ap)
                    nc.vector.tensor_sub(vv, e2_ap, msq)
                    nc.vector.tensor_scalar_add(vv, vv, EPS)
                    nc.vector.reciprocal(rr, vv)
                    nc.scalar.activation(rstd_f, rr, AF.Sqrt)
                    stb = apool.tile([1, 2, T], BF16, tag="stb")
                    rstd_b = stb[:, 0, :]
                    mr_b = stb[:, 1, :]
                    nc.vector.tensor_copy(rstd_b, rstd_f)
                    nc.vector.tensor_mul(mr_b, m_ap, rstd_f)
                    ps_r1 = ps2.tile([128, T], F32, tag="stat")
                    ps_r2 = ps2.tile([128, T], F32, tag="stat")
                    nc.tensor.matmul(ps_r1[:], ones_r[:], rstd_b, start=True, stop=True)
                    nc.tensor.matmul(ps_r2[:], ones_r[:], mr_b, start=True, stop=True)
                    h = apool.tile([128, ND, T], BF16, tag="hn")
                    for d in range(ND):
                        t1 = apool.tile([128, T], F32, tag="t1")
                        nc.vector.tensor_mul(t1[:], x[:, d, :], ps_r1[:])
                        nc.vector.tensor_sub(t1[:], t1[:], ps_r2[:])
                        nc.scalar.activation(h[:, d, :], t1[:], AF.Identity,
                                             bias=lnb_sb[:, l, which, d:d + 1],
                                             scale=lnw_sb[:, l, which, d:d + 1])
                    return h

                def layer(l):
                    h1 = layer_norm(l, 0)

                    wqk_sb = wpool.tile([128, ND, 2 * D], BF16, tag="wqk")
                    for d in range(ND):
                        nc.sync.dma_start(wqk_sb[:, d, :], wqk[l, 128 * d:128 * (d + 1), :])

                    # K^T feature-major [128, ND, T] (features 0..767 = 12 heads x 64)
                    ktl = a1pool.tile([128, ND, T], BF16, tag="ktl")
                    for m in range(6, 12):
                        p = ps.tile([128, 512], F32, tag="gemm")
                        for d in range(ND):
                            nc.tensor.matmul(p[:, 0:T],
                                             wqk_sb[:, d, 128 * m:128 * (m + 1)],
                                             h1[:, d, :],
                                             start=(d == 0), stop=(d == ND - 1))
                        nc.scalar.activation(ktl[:, m - 6, :], p[:, 0:T], AF.Identity,
                                             bias=qkb_sb[:, l, m:m + 1])

                    # V token-major with per-head ones column [128, NT, VA]
                    wv_sb = wpool.tile([128, ND, VA], BF16, tag="wv")
                    for d in range(ND):
                        nc.sync.dma_start(wv_sb[:, d, :], wv[l, 128 * d:128 * (d + 1), :])
                    vb_sb = wpool.tile([128, VA], F32, tag="vb")
                    nc.sync.dma_start(vb_sb[:], vbr[l])
                    vtl = a1pool.tile([128, NT, VA], BF16, tag="vtl")
                    for t in range(NT):
                        for c0, cn in ((0, 512), (512, VA - 512)):
                            p = ps.tile([128, 512], F32, tag="gemm")
                            for d in range(ND):
                                nc.tensor.matmul(p[:, 0:cn],
                                                 h1[:, d, 128 * t:128 * (t + 1)],
                                                 wv_sb[:, d, c0:c0 + cn],
                                                 start=(d == 0), stop=(d == ND - 1))
                            nc.vector.tensor_add(vtl[:, t, c0:c0 + cn], p[:, 0:cn],
                                                 vb_sb[:, c0:c0 + cn])

                    # AllGather K^T and V within the 4-core batch group
                    kvin = dpool.tile([KBYTES + VBYTES], BF16, tag="kvin")
                    kvout = dpool.tile([4, KBYTES + VBYTES], BF16, tag="kvout")
                    nc.sync.dma_start(
                        kvin[0:KBYTES].rearrange("(p f) -> p f", p=128), ktl[:])
                    nc.sync.dma_start(
                        kvin[KBYTES:].rearrange("(p f) -> p f", p=128), vtl[:])
                    nc.gpsimd.collective_compute(
                        "AllGather", mybir.AluOpType.bypass,
                        replica_groups=rg,
                        ins=[kvin.opt()], outs=[kvout.opt()],
                    )

                    # Q^T while AG is in flight
                    qt = a1pool.tile([128, ND, T], BF16, tag="qt")
                    for m in range(6):
                        p = ps.tile([128, 512], F32, tag="gemm")
                        for d in range(ND):
                            nc.tensor.matmul(p[:, 0:T],
                                             wqk_sb[:, d, 128 * m:128 * (m + 1)],
                                             h1[:, d, :],
                                             start=(d == 0), stop=(d == ND - 1))
                        nc.scalar.activation(qt[:, m, :], p[:, 0:T], AF.Identity,
                                             bias=qkb_sb[:, l, m:m + 1])

                    ktall = a1pool.tile([128, 4, ND, T], BF16, tag="ktall")
                    vall = a1pool.tile([128, 4, NT, VA], BF16, tag="vall")
                    for j in range(4):
                        nc.sync.dma_start(
                            ktall[:, j],
                            kvout[j, 0:KBYTES].rearrange("(p a b) -> p a b", p=128, a=ND))
                        nc.sync.dma_start(
                            vall[:, j],
                            kvout[j, KBYTES:].rearrange("(p a b) -> p a b", p=128, a=NT))

                    # attention
                    o_fin = a1pool.tile([128, ND, T], BF16, tag="ofin")
                    dnsb = apool.tile([1, H, T], BF16, tag="dnsb")
                    for hp in range(6):          # head pairs (2hp, 2hp+1)
                        ps_pair = []
                        for hh in range(2):
                            h_ = 2 * hp + hh
                            r0 = hh * 64
                            at = apool.tile([128, NK, T], BF16, tag="at")
                            for k in range(NK):
                                j, kk = k // 2, k % 2
                                p = ps.tile([128, 512], F32, tag="gemm")
                                nc.tensor.matmul(
                                    p[:, 0:T],
                                    ktall[r0:r0 + 64, j, hp, 128 * kk:128 * (kk + 1)],
                                    qt[r0:r0 + 64, hp, :],
                                    start=True, stop=True)
                                nc.scalar.activation(at[:, k, :], p[:, 0:T], AF.Exp,
                                                     scale=0.125)
                                nc.vector.tensor_mul(at[:, k, :], at[:, k, :],
                                                     msk_sb[:, k, :])
                            po = ps.tile([128, 512], F32, tag="gemm")
                            for k in range(NK):
                                j, kk = k // 2, k % 2
                                nc.tensor.matmul(
                                    po[0:65, 0:T],
                                    vall[:, j, kk, 65 * h_:65 * h_ + 65],
                                    at[:, k, :],
                                    start=(k == 0), stop=(k == NK - 1))
                            ps_pair.append(po)
                            # reciprocal of the softmax denominator (row 64)
                            with nc.allow_low_precision(reason="softmax recip bf16"):
                                nc.vector.reciprocal(dnsb[:, h_, :],
                                                     po[64:65, 0:T])
                        # replicate recip across the pair's 128 feature rows
                        pr = ps2.tile([128, T], F32, tag="stat")
                        nc.tensor.matmul(pr[0:64, :], ones_r[:, 0:64],
                                         dnsb[:, 2 * hp, :], start=True, stop=True)
                        nc.tensor.matmul(pr[64:128, :], ones_r[:, 0:64],
                                         dnsb[:, 2 * hp + 1, :], start=True, stop=True)
                        rep = apool.tile([128, T], F32, tag="rep")
                        nc.scalar.activation(rep[:], pr[:], AF.Copy)
                        # even head: rows 0..63 ; odd head: shift copy to 64..127
                        nc.vector.tensor_mul(o_fin[0:64, hp, :], ps_pair[0][0:64, 0:T],
                                             rep[0:64, :])
                        otmp = apool.tile([128, T], F32, tag="otmp")
                        nc.scalar.activation(otmp[64:128, :], ps_pair[1][0:64, 0:T],
                                             AF.Copy)
                        nc.vector.tensor_mul(o_fin[64:128, hp, :], otmp[64:128, :],
                                             rep[64:128, :])

                    # attention out-projection + residual
                    wo_sb = wpool.tile([128, ND, D], BF16, tag="wo")
                    for d in range(ND):
                        nc.sync.dma_start(wo_sb[:, d, :], wo[l, 128 * d:128 * (d + 1), :])
                    for m in range(ND):
                        p = ps.tile([128, 512], F32, tag="gemm")
                        for d in range(ND):
                            nc.tensor.matmul(p[:, 0:T],
                                             wo_sb[:, d, 128 * m:128 * (m + 1)],
                                             o_fin[:, d, :],
                                             start=(d == 0), stop=(d == ND - 1))
                        tr = apool.tile([128, T], F32, tag="tres")
                        nc.scalar.activation(tr[:], p[:, 0:T], AF.Identity,
                                             bias=aob_sb[:, l, m:m + 1])
                        nc.vector.tensor_add(x[:, m, :], x[:, m, :], tr[:])

                    # FFN
                    h2 = layer_norm(l, 1)
                    g = a1pool.tile([128, NF, T], BF16, tag="g")
                    for blk in range(4):
                        w1c = w2pool.tile([128, ND, 768], BF16, tag="w1c", bufs=2)
                        for d in range(ND):
                            nc.sync.dma_start(
                                w1c[:, d, :],
                                w1[l, 128 * d:128 * (d + 1), 768 * blk:768 * (blk + 1)])
                        for mm in range(6):
                            m = 6 * blk + mm
                            p = ps.tile([128, 512], F32, tag="gemm")
                            for d in range(ND):
                                nc.tensor.matmul(p[:, 0:T],
                                                 w1c[:, d, 128 * mm:128 * (mm + 1)],
                                                 h2[:, d, :],
                                                 start=(d == 0), stop=(d == ND - 1))
                            nc.scalar.activation(g[:, m, :], p[:, 0:T], AF.Gelu,
                                                 bias=b1p_sb[:, l, m:m + 1])
                    ps_f2 = [ps.tile([128, 512], F32, tag="gemm", name=f"psf2_{m}")
                             for m in range(ND)]
                    for d2 in range(NF):
                        w2c = w2pool.tile([128, D], BF16, tag="w2c")
                        nc.sync.dma_start(w2c[:], w2[l, 128 * d2:128 * (d2 + 1), :])
                        for m in range(ND):
                            nc.tensor.matmul(ps_f2[m][:, 0:T],
                                             w2c[:, 128 * m:128 * (m + 1)],
                                             g[:, d2, :],
                                             start=(d2 == 0), stop=(d2 == NF - 1))
                    for m in range(ND):
                        tr = apool.tile([128, T], F32, tag="tres")
                        nc.scalar.activation(tr[:], ps_f2[m][:, 0:T], AF.Identity,
                                             bias=b2p_sb[:, l, m:m + 1])
                        nc.vector.tensor_add(x[:, m, :], x[:, m, :], tr[:])

                for l in range(n_layers):
                    layer(l)
                nc.sync.dma_start(xdbg[:], x[:])

            # ---- vocab head ----
            with (
                tc.tile_pool(name="hd", bufs=1) as hpool,
                tc.tile_pool(name="pwp", bufs=24) as pwpool,
                tc.tile_pool(name="hsb", bufs=4) as hspool,
                tc.tile_pool(name="hps", bufs=6, space="PSUM") as hps,
            ):
                xbf = hpool.tile([128, ND, T], BF16, tag="xbf")
                for d in range(ND):
                    nc.scalar.activation(xbf[:, d, :], x[:, d, :], AF.Copy)
                sacc = hpool.tile([128, NT], F32, tag="sacc")
                nc.vector.memset(sacc[:], 0.0)

                for vg in range(21):
                    cs = [3 * vg, 3 * vg + 1, 3 * vg + 2]
                    psl = [hps.tile([128, 512], F32, tag="head", name=f"psl_{i}")
                           for i in range(6)]
                    pwc = {}
                    for d in range(ND):
                        for ci, c in enumerate(cs):
                            cn = _vc_width(c)
                            w = pwpool.tile([128, 512], BF16, tag="pw")
                            nc.sync.dma_start(w[:, 0:cn],
                                              pw[128 * d:128 * (d + 1),
                                                 512 * c:512 * c + cn])
                            pwc[(d, ci)] = w
                        for t in range(NT):
                            for ci, c in enumerate(cs):
                                cn = _vc_width(c)
                                nc.tensor.matmul(
                                    psl[ci * NT + t][:, 0:cn],
                                    xbf[:, d, 128 * t:128 * (t + 1)],
                                    pwc[(d, ci)][:, 0:cn],
                                    start=(d == 0), stop=(d == ND - 1))
                    for ci, c in enumerate(cs):
                        cn = _vc_width(c)
                        for t in range(NT):
                            p = psl[ci * NT + t]
                            lsb = hspool.tile([128, 512], F32, tag="lsb")
                            if include_pb:
                                pbc = hspool.tile([128, 512], F32, tag="pbc")
                                nc.sync.dma_start(pbc[:, 0:cn],
                                                  pbr[:, 512 * c:512 * c + cn])
                                nc.vector.tensor_add(lsb[:, 0:cn], p[:, 0:cn],
                                                     pbc[:, 0:cn])
                            else:
                                nc.vector.tensor_copy(lsb[:, 0:cn], p[:, 0:cn])
                            nc.sync.dma_start(
                                logits[128 * t:128 * (t + 1), 512 * c:512 * c + cn],
                                lsb[:, 0:cn])
                            esb = hspool.tile([128, 512], F32, tag="esb")
                            nc.scalar.activation(esb[:, 0:cn], lsb[:, 0:cn], AF.Exp)
                            red = hspool.tile([128, 1], F32, tag="red")
                            nc.vector.reduce_sum(red[:], esb[:, 0:cn], axis=AX.X)
                            nc.vector.tensor_add(sacc[:, t:t + 1], sacc[:, t:t + 1],
                                                 red[:])
                for t in range(NT):
                    nc.sync.dma_start(sumexp[t, :].rearrange("(a p) -> p a", a=1),
                                      sacc[:, t:t + 1])

    nc.compile()
    _NC_CACHE[key] = nc
    return nc


# ----------------------------------------------------------------------------
# host-side input preparation
# ----------------------------------------------------------------------------

def _rotary_np(x):
    """x [B, S, D] f32 -> rotary applied to first D//2 dims (pairs)."""
    rd = D // 2
    freqs = 1.0 / (10000.0 ** (np.arange(0, rd, 2, dtype=np.float32) / rd))
    ang = np.arange(S, dtype=np.float32)[:, None] * freqs[None, :]
    cos, sin = np.cos(ang), np.sin(ang)          # [S, rd//2]
    xr = x[..., :rd].reshape(B, S, rd // 2, 2)
    x0, x1 = xr[..., 0], xr[..., 1]
    r0 = x0 * cos - x1 * sin
    r1 = x0 * sin + x1 * cos
    rot = np.stack([r0, r1], axis=-1).reshape(B, S, rd)
    return np.concatenate([rot, x[..., rd:]], axis=-1)


def _fm(a):
    """[T?, 768] -> feature-major [128, 6, T?] (f32)."""
    return np.ascontiguousarray(a.T.reshape(ND, 128, a.shape[0]).transpose(1, 0, 2))


def _bf(a):
    return np.ascontiguousarray(a).astype(ml_dtypes.bfloat16)


def prep_inputs(input_ids, target_ids, noise_level, emb, in_w, in_b,
                attn_out_w, attn_out_b, ln1_w, ln1_b, ln2_w, ln2_b,
                w1, b1, w2, b2, proj_w, proj_b):
    input_ids = np.asarray(input_ids)
    emb = np.asarray(emb, np.float32)
    xall = emb[input_ids]                       # [B, S, D]
    xall = _rotary_np(xall)

    include_pb = bool(np.any(np.asarray(proj_b)))

    wqk_h = np.empty((L, D, 2 * D), ml_dtypes.bfloat16)
    wv_h = np.zeros((L, D, VA), ml_dtypes.bfloat16)
    vbr_h = np.zeros((L, 128, VA), np.float32)
    wo_h = np.empty((L, D, D), ml_dtypes.bfloat16)
    w1_h = np.empty((L, D, DF), ml_dtypes.bfloat16)
    w2_h = np.empty((L, DF, D), ml_dtypes.bfloat16)
    in_w = np.asarray(in_w, np.float32)
    in_b = np.asarray(in_b, np.float32)
    for l in range(L):
        wqk_h[l] = _bf(in_w[l, :2 * D, :].T)
        wvT = in_w[l, 2 * D:, :].T              # [768(d), 768(v feature)]
        for h in range(H):
            wv_h[l, :, 65 * h:65 * h + 64] = _bf(wvT[:, 64 * h:64 * (h + 1)])
            vbr_h[l, :, 65 * h:65 * h + 64] = in_b[l, 2 * D + 64 * h:2 * D + 64 * (h + 1)][None, :]
            vbr_h[l, :, 65 * h + 64] = 1.0
        wo_h[l] = _bf(np.asarray(attn_out_w[l], np.float32).T)
        w1_h[l] = _bf(np.asarray(w1[l], np.float32).T)
        w2_h[l] = _bf(np.asarray(w2[l], np.float32).T)
    pw_h = _bf(np.asarray(proj_w, np.float32).T)          # [768, 32000]

    def perp(v):                                # [L, 768] -> [128, L, 6]
        v = np.asarray(v, np.float32).reshape(L, ND, 128)
        return np.ascontiguousarray(v.transpose(2, 0, 1))

    lnw_h = np.stack([perp(ln1_w), perp(ln2_w)], axis=2)  # [128, L, 2, 6]
    lnb_h = np.stack([perp(ln1_b), perp(ln2_b)], axis=2)
    qkb_h = np.ascontiguousarray(
        np.asarray(in_b, np.float32)[:, :2 * D].reshape(L, 12, 128).transpose(2, 0, 1))
    aob_h = perp(attn_out_b)
    b1p_h = np.ascontiguousarray(
        np.asarray(b1, np.float32).reshape(L, NF, 128).transpose(2, 0, 1))
    b2p_h = perp(b2)

    sel_h = np.zeros((H, D), ml_dtypes.bfloat16)
    for h in range(H):
        sel_h[h, 64 * h:64 * (h + 1)] = 1.0

    in_maps = []
    for c in range(N_CORES):
        b, r = c // 4, c % 4
        s0 = T * r
        xc = _fm(xall[b, s0:s0 + T, :].astype(np.float32))
        # mask[128k+kk, q]: key block <= query block
        keys = np.arange(S)[:, None] // BS
        qs = (s0 + np.arange(T))[None, :] // BS
        m = (keys <= qs).astype(np.float32)
        msk_h = np.ascontiguousarray(
            m.reshape(NK, 128, T).transpose(1, 0, 2)).astype(ml_dtypes.bfloat16)
        im = dict(x0=xc, wqk=wqk_h, wv=wv_h, vbr=vbr_h, wo=wo_h, w1=w1_h,
                  w2=w2_h, pw=pw_h, lnw=lnw_h, lnb=lnb_h, qkb=qkb_h,
                  aob=aob_h, b1p=b1p_h, b2p=b2p_h, msk=msk_h, sel=sel_h)
        if include_pb:
            im["pbr"] = np.ascontiguousarray(
                np.broadcast_to(np.asarray(proj_b, np.float32)[None, :], (128, V)))
        in_maps.append(im)
    return in_maps, include_pb


def run_cores(inputs, n_layers=N_LAYERS, trace=False):
    in_maps, include_pb = prep_inputs(**inputs)
    nc = build_nc(n_layers, include_pb)
    res = run_bass_kernel_spmd(nc, in_maps, list(range(N_CORES)), trace=trace)
    return res


def timed_run(inputs, n_layers=N_LAYERS, iters=5):
    """Wall-clock timing with device-resident inputs (no NTFF in this env)."""
    import time
    import jax
    from jax.sharding import Mesh, PartitionSpec
    from jax.experimental.shard_map import shard_map
    from concourse import bass2jax, mybir as _mb

    in_maps, include_pb = prep_inputs(**inputs)
    nc = build_nc(n_layers, include_pb)
    bass2jax.install_neuronx_cc_hook()

    partition_name = nc.partition_id_tensor.name if nc.partition_id_tensor else None
    in_names, out_names, out_avals, zero_outs = [], [], [], []
    for alloc in nc.m.functions[0].allocations:
        if not isinstance(alloc, _mb.MemoryLocationSet):
            continue
        name = alloc.memorylocations[0].name
        if alloc.kind == "ExternalInput":
            if name != partition_name:
                in_names.append(name)
        elif alloc.kind == "ExternalOutput":
            shape = tuple(alloc.tensor_shape)
            dtype = _mb.dt.np(alloc.dtype)
            out_avals.append(jax.core.ShapedArray(shape, dtype))
            out_names.append(name)
            zero_outs.append(np.zeros(shape, dtype))
    n_params = len(in_names)
    in_names_all = in_names + out_names
    if partition_name is not None:
        in_names_all = in_names_all + [partition_name]

    def _body(*args):
        operands = list(args)
        if partition_name is not None:
            operands.append(bass2jax.partition_id_tensor())
        outs = bass2jax._bass_exec_p.bind(
            *operands, out_avals=tuple(out_avals),
            in_names=tuple(in_names_all), out_names=tuple(out_names),
            lowering_input_output_aliases=(),
            sim_require_finite=True, sim_require_nnan=True, nc=nc)
        return tuple(outs)

    devices = jax.devices()[:N_CORES]
    mesh = Mesh(np.asarray(devices), ("core",))
    nin = n_params + len(zero_outs)
    f = jax.jit(shard_map(_body, mesh=mesh,
                          in_specs=(PartitionSpec("core"),) * nin,
                          out_specs=(PartitionSpec("core"),) * len(out_names),
                          check_rep=False), keep_unused=True)
    per_core = [[np.asarray(m[nm]) for nm in in_names] for m in in_maps]
    concat_in = [np.concatenate([per_core[c][i] for c in range(N_CORES)], axis=0)
                 for i in range(n_params)]
    concat_zeros = [np.zeros((N_CORES * z.shape[0], *z.shape[1:]), z.dtype)
                    for z in zero_outs]
    sh = jax.sharding.NamedSharding(mesh, PartitionSpec("core"))
    dev_in = [jax.device_put(a, sh) for a in concat_in + concat_zeros]
    out = f(*dev_in)
    jax.block_until_ready(out)
    times = []
    for _ in range(iters):
        t0 = time.perf_counter()
        out = f(*dev_in)
        jax.block_until_ready(out)
        times.append(time.perf_counter() - t0)
    return times


def kernel(**inputs):
    res = run_cores(inputs)
    input_ids = np.asarray(inputs["input_ids"])
    target_ids = np.asarray(inputs["target_ids"])
    noise_level = np.asarray(inputs["noise_level"], np.float32)

    logits = np.empty((B, S, V), np.float32)
    sumexp = np.empty((B, S), np.float32)
    for c in range(N_CORES):
        b, r = c // 4, c % 4
        s0 = T * r
        logits[b, s0:s0 + T] = res.results[c]["logits"]
        sumexp[b, s0:s0 + T] = res.results[c]["sumexp"].reshape(S // 4 // 128, 128).reshape(-1)

    lse = np.log(sumexp)                                   # [B, S]
    tgt = np.take_along_axis(logits, target_ids[..., None].astype(np.int64),
                             axis=-1)[..., 0]
    ce = lse - tgt
    is_masked = (input_ids == MASK_ID).astype(np.float32)
    weighted = ce * is_masked / noise_level[:, None]
    loss = np.float32(weighted.sum() / (is_masked.sum() + 1e-8))
    return loss, logits
